# revision 14
# baseline (speedup 1.0000x reference)
"""CrossSetNorm Trainium2 kernel v3 (8 NeuronCores, batch-parallel).

Problem: x [2048, 328, 256] f32, mask [2048, 328] bool (True = dead).
Two independent masked set-norms over the set dim per sample:
  obj = s in [0, 128), road = s in [128, 328)
out[s,d] = alive_s * x[s,d] * A[d] + C[d] with A = istd*w, C = b - mean*A.

Design (per core, 256 samples):
  - HOST: pre-mask xm = x*alive, cast bf16, transpose to [grp, d, samp, s]
    so the set dim s lands on the SBUF free axis and d on partitions.
    Output returns transposed bf16 and is un-transposed + cast on host.
  - stats: one DVE bn_stats per (sample, seg, d-half) gives (count, mean,
    M2) for even/odd elements -> s1/s2 recovered in a per-superchunk
    (64 samples) chain on [128, 2, 64] tiles (d on partitions,
    (half, sample) on free).
  - count-derived scalars (rcnt, okt, okm, ...) are computed per
    32-sample subchunk in [32, 2] sample-space, packed, stream-
    transposed, and broadcast to 128 partitions with one-hot-selector
    PE matmuls into PSUM.
  - istd = Exp(-0.5*Ln(var+eps)) on ScalarE; A/C tiles [128, 64] per
    (seg, half) hold per-sample columns.
  - apply: in-place per (sample, seg, half): DVE tensor_scalar
    (xm*A + C) or ScalarE activation Identity(scale=A, bias=C), split
    across both engines to balance load. Out-DMA per tile via gpsimd
    SWDGE; no gpsimd compute (the v1 bottleneck: gpsimd 89% busy).
"""
import sys

if "/opt/trn_rl_repo" not in sys.path:
    sys.path.insert(0, "/opt/trn_rl_repo")

from contextlib import ExitStack

import numpy as np
import ml_dtypes

import concourse.bacc as bacc
import concourse.bass as bass
import concourse.tile as tile
from concourse import mybir
from concourse.bass_utils import run_bass_kernel_spmd

F32 = mybir.dt.float32
F32R = mybir.dt.float32r
BF16 = mybir.dt.bfloat16
AF = mybir.ActivationFunctionType
OP = mybir.AluOpType
AX = mybir.AxisListType
BF = ml_dtypes.bfloat16

NCORES = 8
B, S, D = 2048, 328, 256
B_LOC = B // NCORES      # 256
S_OBJ = 128
S_ROAD = S - S_OBJ       # 200
G = 8                    # samples per xg tile
NGRP = B_LOC // G        # 32 groups per core
CHUNK = 32               # samples per stats chunk
SUB = 32                 # samples per count subchunk
NSUB = CHUNK // SUB      # 2
GPC = CHUNK // G         # 8 groups per superchunk
NCHUNK = B_LOC // CHUNK  # 4
EPS = 1e-6
# p3 road ops also routed to ScalarE when (bcol % 16) < ROAD_SC
ROAD_SC = 10

_NC_CACHE = {}


def build_nc():
    nc = bacc.Bacc("TRN2", target_bir_lowering=False, debug=False, num_devices=NCORES)
    x_d = nc.declare_dram_parameter("x", [NGRP, D, G // 2, S, 2], BF16, isOutput=False)
    alive_d = nc.declare_dram_parameter("alive", [B_LOC, S], BF16, isOutput=False)
    w_obj_d = nc.declare_dram_parameter("weights_obj", [D], F32, isOutput=False)
    b_obj_d = nc.declare_dram_parameter("biases_obj", [D], F32, isOutput=False)
    w_road_d = nc.declare_dram_parameter("weights_road", [D], F32, isOutput=False)
    b_road_d = nc.declare_dram_parameter("biases_road", [D], F32, isOutput=False)
    sel_d = nc.declare_dram_parameter("sel", [10, 10 * 128], F32R, isOutput=False)
    out_d = nc.declare_dram_parameter("out", [NGRP, D, G // 2, S, 2], BF16, isOutput=True)

    with tile.TileContext(nc) as tc, ExitStack() as ctx:
        singles = ctx.enter_context(tc.tile_pool(name="singles", bufs=1))
        chunkp = ctx.enter_context(tc.tile_pool(name="chunkp", bufs=2))
        xp = ctx.enter_context(tc.tile_pool(name="xp", bufs=24))
        bnp = ctx.enter_context(tc.tile_pool(name="bnp", bufs=2))
        psum = ctx.enter_context(tc.tile_pool(name="psum", bufs=2, space="PSUM"))

        # ---- constants ----
        eps_col = singles.tile([128, 1], F32)
        nc.vector.memset(eps_col, EPS)
        # one-hot selector: SEL[k, 128*r + m] = (k == r); lhsT slice r picks
        # packT row r and broadcasts it to 128 partitions (K=10 matmul)
        SEL = singles.tile([10, 10 * 128], F32R)
        nc.sync.dma_start(out=SEL, in_=sel_d[:, :])
        nhalf = singles.tile([SUB, 2], F32)  # even-element counts per seg
        nc.vector.memset(nhalf[:, 0:1], float(S_OBJ // 2))
        nc.vector.memset(nhalf[:, 1:2], float(S_ROAD // 2))
        nfull = singles.tile([SUB, 2], F32)  # segment sizes
        nc.vector.memset(nfull[:, 0:1], float(S_OBJ))
        nc.vector.memset(nfull[:, 1:2], float(S_ROAD))
        wb = {}
        for seg, (wd, bd) in enumerate(((w_obj_d, b_obj_d), (w_road_d, b_road_d))):
            for h in range(2):
                wt = singles.tile([128, 1], F32, name=f"w{seg}{h}")
                nc.sync.dma_start(
                    out=wt, in_=bass.AP(tensor=wd, offset=128 * h, ap=[[1, 128], [1, 1]])
                )
                bt = singles.tile([128, 1], F32, name=f"b{seg}{h}")
                nc.sync.dma_start(
                    out=bt, in_=bass.AP(tensor=bd, offset=128 * h, ap=[[1, 128], [1, 1]])
                )
                wb[(seg, h)] = (wt, bt)

        xg_tiles = {}
        av_tiles = {}

        def emit_loads(c):
            av = chunkp.tile([CHUNK, S], BF16, name="av")
            nc.sync.dma_start(out=av, in_=alive_d[c * CHUNK : (c + 1) * CHUNK, :])
            av_tiles[c] = av
            for g in range(GPC):
                grp = c * GPC + g
                for h in range(2):
                    t = xp.tile([128, G // 2, S, 2], BF16, tag="xg", name="xg")
                    nc.sync.dma_start(out=t, in_=x_d[grp, 128 * h : 128 * (h + 1), :, :, :])
                    xg_tiles[(grp, h)] = t

        emit_loads(0)
        for c in range(NCHUNK):
            if c + 1 < NCHUNK:
                emit_loads(c + 1)

            # ---- count-derived rows per subchunk -> PE broadcast into Rps ----
            av = av_tiles.pop(c)
            Rps = psum.tile([128, 5, 2, 2, CHUNK], F32, tag="bank", name="Rps")
            for sub in range(NSUB):
                r0 = sub * SUB
                cnt = chunkp.tile([SUB, 2], F32, name=f"cnt{sub}")
                nc.vector.tensor_reduce(
                    cnt[:, 0:1], av[r0 : r0 + SUB, 0:S_OBJ], AX.X, OP.add
                )
                nc.vector.tensor_reduce(
                    cnt[:, 1:2], av[r0 : r0 + SUB, S_OBJ:S], AX.X, OP.add
                )
                cc = chunkp.tile([SUB, 2], F32, name=f"cc{sub}")
                rc = chunkp.tile([SUB, 2], F32, name=f"rc{sub}")
                pack = chunkp.tile([SUB, 32], F32, name=f"pack{sub}")
                nc.vector.tensor_scalar(cc, cnt, 1.0, None, OP.max)
                nc.vector.reciprocal(rc, cc)
                # rows: 0,1 = nr1 = nhalf*rcnt; 2,3 = g = nfull*rcnt - 2;
                #       4,5 = rcnt; 6,7 = okt; 8,9 = okm   (cols = obj, road)
                nc.vector.tensor_mul(pack[:, 0:2], rc, nfull)
                nc.vector.tensor_mul(pack[:, 2:4], rc, nfull)
                nc.vector.tensor_scalar(pack[:, 2:4], pack[:, 2:4], -2.0, None, OP.add)
                nc.vector.tensor_scalar(pack[:, 4:6], rc, 1.0, None, OP.mult)
                nc.vector.tensor_scalar(pack[:, 6:8], cnt, -1.0, 1.0, OP.add, OP.min)
                nc.vector.tensor_scalar(pack[:, 6:8], pack[:, 6:8], 0.0, None, OP.max)
                nc.vector.tensor_scalar(
                    pack[:, 8:10], pack[:, 6:8], -1.0, 1.0, OP.mult, OP.add
                )
                packT = chunkp.tile([SUB, 32], F32, name=f"packT{sub}")
                nc.vector.transpose(packT, pack)
                packR = chunkp.tile([SUB, 32], F32R, name=f"packR{sub}")
                nc.vector.tensor_scalar(packR, packT, 1.0, None, OP.mult)
                for r in range(5):
                    for seg in range(2):
                        k = 2 * r + seg
                        for h in range(2):
                            nc.tensor.matmul(
                                Rps[:, r : r + 1, seg : seg + 1, h : h + 1,
                                    r0 : r0 + SUB],
                                SEL[:, 128 * k : 128 * (k + 1)],
                                packR[0:10, :],
                                start=True, stop=True,
                            )
            Rsb = chunkp.tile([128, 5, 2, 2, CHUNK], F32, name="Rsb")
            nc.vector.tensor_scalar(Rsb, Rps, 1.0, None, OP.mult)

            # ---- bn_stats over the set dim (d on partitions) ----
            BNo = bnp.tile([128, 2, CHUNK // 2, 2, 3], F32, tag="bno", name="BNo")
            BNr = bnp.tile([128, 2, CHUNK // 2, 2, 3], F32, tag="bnr", name="BNr")
            for g in range(GPC):
                grp = c * GPC + g
                for h in range(2):
                    xg = xg_tiles[(grp, h)]
                    for p in range(G // 2):  # 2 interleaved samples per op
                        pr = (G // 2) * g + p
                        nc.vector.bn_stats(
                            BNo[:, h : h + 1, pr : pr + 1, :, :],
                            xg[:, p : p + 1, 0:S_OBJ, :].rearrange(
                                "p a s t -> p a (s t)"
                            ),
                        )
                        nc.vector.bn_stats(
                            BNr[:, h : h + 1, pr : pr + 1, :, :],
                            xg[:, p : p + 1, S_OBJ:S, :].rearrange(
                                "p a s t -> p a (s t)"
                            ),
                        )

            # ---- stats chain per segment on [128, 2, 64] ----
            ACs = {}
            for seg, BN in ((0, BNo), (1, BNr)):
                me, Me = BN[:, :, :, :, 1:2], BN[:, :, :, :, 2:3]

                def row(r, _seg=seg):
                    return Rsb[:, r : r + 1, _seg : _seg + 1, :, :]

                U = chunkp.tile([128, 2, CHUNK], F32, name=f"U{seg}")
                MEAN = chunkp.tile([128, 2, CHUNK], F32, name=f"MEAN{seg}")
                A1 = chunkp.tile([128, 2, CHUNK], F32, name=f"A1{seg}")
                B1 = chunkp.tile([128, 2, CHUNK], F32, name=f"B1{seg}")
                ISTD = chunkp.tile([128, 2, CHUNK], F32, name=f"ISTD{seg}")
                nc.vector.tensor_mul(MEAN, me, row(0))     # mean = me*n*rcnt
                nc.vector.tensor_mul(A1, me, me)
                nc.vector.tensor_mul(A1, A1, row(0))       # me^2*n*rcnt
                nc.vector.tensor_mul(B1, Me, row(2))       # M2*rcnt
                nc.vector.tensor_add(U, A1, B1)            # s2/cnt
                nc.vector.tensor_mul(B1, MEAN, MEAN)       # mean^2
                nc.vector.tensor_mul(B1, B1, row(1))       # *(n*rcnt-2)
                nc.vector.tensor_add(U, U, B1)             # var
                nc.vector.tensor_scalar(U, U, EPS, None, OP.add)
                nc.vector.reciprocal(B1, U)                # 1/(var+eps)
                nc.scalar.activation(ISTD, B1, AF.Sqrt)    # istd
                nc.vector.tensor_mul(ISTD, ISTD, row(3))   # *okt
                nc.vector.tensor_add(ISTD, ISTD, row(4))   # +okm -> istd_eff
                for h in range(2):
                    wt, bt = wb[(seg, h)]
                    A_t = chunkp.tile([128, CHUNK], F32, name=f"A{seg}{h}")
                    C_t = chunkp.tile([128, CHUNK], F32, name=f"C{seg}{h}")
                    nc.vector.tensor_scalar(
                        A_t, ISTD[:, h : h + 1, :], wt[:, :], None, OP.mult
                    )
                    nc.vector.tensor_mul(C_t, MEAN[:, h : h + 1, :], A_t)
                    nc.vector.tensor_scalar(C_t, C_t, -1.0, bt[:, :], OP.mult, OP.add)
                    ACs[(seg, h)] = (A_t, C_t)

            # ---- apply in-place + out DMA ----
            for g in range(GPC):
                grp = c * GPC + g
                for h in range(2):
                    xg = xg_tiles.pop((grp, h))
                    Ao, Co = ACs[(0, h)]
                    Ar, Cr = ACs[(1, h)]
                    last = c == NCHUNK - 1
                    for j in range(G):
                        bcol = G * g + j
                        p, t = j // 2, j % 2
                        # obj: ScalarE (last chunk: half to DVE to kill tail)
                        o_sl = xg[:, p : p + 1, 0:S_OBJ, t : t + 1]
                        if last and bcol % 2 == 1:
                            nc.vector.tensor_scalar(
                                o_sl, o_sl,
                                Ao[:, bcol : bcol + 1], Co[:, bcol : bcol + 1],
                                OP.mult, OP.add,
                            )
                        else:
                            nc.scalar.activation(
                                o_sl, o_sl,
                                AF.Identity,
                                bias=Co[:, bcol : bcol + 1],
                                scale=Ao[:, bcol : bcol + 1],
                            )
                        # road: split DVE / ScalarE
                        r_sl = xg[:, p : p + 1, S_OBJ:S, t : t + 1]
                        road_sc = 4 if last else ROAD_SC
                        if bcol % 16 < road_sc:
                            nc.scalar.activation(
                                r_sl, r_sl,
                                AF.Identity,
                                bias=Cr[:, bcol : bcol + 1],
                                scale=Ar[:, bcol : bcol + 1],
                            )
                        else:
                            nc.vector.tensor_scalar(
                                r_sl, r_sl,
                                Ar[:, bcol : bcol + 1], Cr[:, bcol : bcol + 1],
                                OP.mult, OP.add,
                            )
                    nc.gpsimd.dma_start(
                        out=out_d[grp, 128 * h : 128 * (h + 1), :, :, :],
                        in_=xg[:, :, :, :],
                    )

    nc.compile()
    return nc


def _get_nc():
    if "nc" not in _NC_CACHE:
        _NC_CACHE["nc"] = build_nc()
    return _NC_CACHE["nc"]


def kernel(x, mask, weights_obj, biases_obj, weights_road, biases_road, _trace=False):
    x = np.asarray(x, dtype=np.float32)
    mask = np.asarray(mask).astype(bool)
    w_obj = np.ascontiguousarray(np.asarray(weights_obj, dtype=np.float32))
    b_obj = np.ascontiguousarray(np.asarray(biases_obj, dtype=np.float32))
    w_road = np.ascontiguousarray(np.asarray(weights_road, dtype=np.float32))
    b_road = np.ascontiguousarray(np.asarray(biases_road, dtype=np.float32))

    xm = np.where(mask[:, :, None], np.float32(0), x).astype(BF)
    # [8, NGRP, G//2, 2, S, D] -> [8, NGRP, D, G//2, S, 2]
    xt = np.ascontiguousarray(
        xm.reshape(NCORES, NGRP, G // 2, 2, S, D).transpose(0, 1, 5, 2, 4, 3)
    )
    alive = (~mask).astype(BF).reshape(NCORES, B_LOC, S)
    sel = np.zeros((10, 10 * 128), dtype=np.float32)
    for r in range(10):
        sel[r, 128 * r : 128 * (r + 1)] = 1.0

    in_maps = [
        {
            "x": xt[i],
            "alive": alive[i],
            "sel": sel,
            "weights_obj": w_obj,
            "biases_obj": b_obj,
            "weights_road": w_road,
            "biases_road": b_road,
        }
        for i in range(NCORES)
    ]
    nc = _get_nc()
    res = run_bass_kernel_spmd(nc, in_maps, core_ids=list(range(NCORES)), trace=_trace)
    outs = []
    for i in range(NCORES):
        o = np.asarray(res.results[i]["out"])  # [NGRP, D, G//2, S, 2] bf16
        outs.append(o.transpose(0, 2, 4, 3, 1).reshape(B_LOC, S, D))
    out = np.concatenate(outs, axis=0).astype(np.float32)
    if _trace:
        kernel.last_exec_time_ns = res.exec_time_ns
        kernel.last_mean_exec_time_ns = res.mean_exec_time_ns
    return out.reshape(B, S, D)


# revision 16
# speedup vs baseline: 1.1361x; 1.1361x over previous
"""CrossSetNorm Trainium2 kernel v3 (8 NeuronCores, batch-parallel).

Problem: x [2048, 328, 256] f32, mask [2048, 328] bool (True = dead).
Two independent masked set-norms over the set dim per sample:
  obj = s in [0, 128), road = s in [128, 328)
out[s,d] = alive_s * x[s,d] * A[d] + C[d] with A = istd*w, C = b - mean*A.

Design (per core, 256 samples):
  - HOST: pre-mask xm = x*alive, cast bf16, transpose to [grp, d, samp, s]
    so the set dim s lands on the SBUF free axis and d on partitions.
    Output returns transposed bf16 and is un-transposed + cast on host.
  - stats: one DVE bn_stats per (sample, seg, d-half) gives (count, mean,
    M2) for even/odd elements -> s1/s2 recovered in a per-superchunk
    (64 samples) chain on [128, 2, 64] tiles (d on partitions,
    (half, sample) on free).
  - count-derived scalars (rcnt, okt, okm, ...) are computed per
    32-sample subchunk in [32, 2] sample-space, packed, stream-
    transposed, and broadcast to 128 partitions with one-hot-selector
    PE matmuls into PSUM.
  - istd = Exp(-0.5*Ln(var+eps)) on ScalarE; A/C tiles [128, 64] per
    (seg, half) hold per-sample columns.
  - apply: in-place per (sample, seg, half): DVE tensor_scalar
    (xm*A + C) or ScalarE activation Identity(scale=A, bias=C), split
    across both engines to balance load. Out-DMA per tile via gpsimd
    SWDGE; no gpsimd compute (the v1 bottleneck: gpsimd 89% busy).
"""
import sys

if "/opt/trn_rl_repo" not in sys.path:
    sys.path.insert(0, "/opt/trn_rl_repo")

from contextlib import ExitStack

import numpy as np
import ml_dtypes

import concourse.bacc as bacc
import concourse.bass as bass
import concourse.tile as tile
from concourse import mybir
from concourse.bass_utils import run_bass_kernel_spmd

F32 = mybir.dt.float32
F32R = mybir.dt.float32r
BF16 = mybir.dt.bfloat16
AF = mybir.ActivationFunctionType
OP = mybir.AluOpType
AX = mybir.AxisListType
BF = ml_dtypes.bfloat16

NCORES = 8
B, S, D = 2048, 328, 256
B_LOC = B // NCORES      # 256
S_OBJ = 128
S_ROAD = S - S_OBJ       # 200
G = 8                    # samples per xg tile
NGRP = B_LOC // G        # 32 groups per core
CHUNK = 32               # samples per stats chunk
SUB = 32                 # samples per count subchunk
NSUB = CHUNK // SUB      # 2
GPC = CHUNK // G         # 8 groups per superchunk
NCHUNK = B_LOC // CHUNK  # 4
EPS = 1e-6
# p3 road ops also routed to ScalarE when (bcol % 16) < ROAD_SC
ROAD_SC = 12

_NC_CACHE = {}


def build_nc():
    nc = bacc.Bacc("TRN2", target_bir_lowering=False, debug=False, num_devices=NCORES)
    x_d = nc.declare_dram_parameter("x", [NGRP, D, G, S], BF16, isOutput=False)
    alive_d = nc.declare_dram_parameter("alive", [B_LOC, S], BF16, isOutput=False)
    w_obj_d = nc.declare_dram_parameter("weights_obj", [D], F32, isOutput=False)
    b_obj_d = nc.declare_dram_parameter("biases_obj", [D], F32, isOutput=False)
    w_road_d = nc.declare_dram_parameter("weights_road", [D], F32, isOutput=False)
    b_road_d = nc.declare_dram_parameter("biases_road", [D], F32, isOutput=False)
    sel_d = nc.declare_dram_parameter("sel", [10, 10 * 128], F32R, isOutput=False)
    out_d = nc.declare_dram_parameter("out", [NGRP, D, G, S], BF16, isOutput=True)

    with tile.TileContext(nc) as tc, ExitStack() as ctx:
        singles = ctx.enter_context(tc.tile_pool(name="singles", bufs=1))
        chunkp = ctx.enter_context(tc.tile_pool(name="chunkp", bufs=2))
        xp = ctx.enter_context(tc.tile_pool(name="xp", bufs=24))
        bnp = ctx.enter_context(tc.tile_pool(name="bnp", bufs=2))
        psum = ctx.enter_context(tc.tile_pool(name="psum", bufs=2, space="PSUM"))

        # ---- constants ----
        eps_col = singles.tile([128, 1], F32)
        nc.vector.memset(eps_col, EPS)
        # one-hot selector: SEL[k, 128*r + m] = (k == r); lhsT slice r picks
        # packT row r and broadcasts it to 128 partitions (K=10 matmul)
        SEL = singles.tile([10, 10 * 128], F32R)
        nc.sync.dma_start(out=SEL, in_=sel_d[:, :])
        nhalf = singles.tile([SUB, 2], F32)  # even-element counts per seg
        nc.vector.memset(nhalf[:, 0:1], float(S_OBJ // 2))
        nc.vector.memset(nhalf[:, 1:2], float(S_ROAD // 2))
        nfull = singles.tile([SUB, 2], F32)  # segment sizes
        nc.vector.memset(nfull[:, 0:1], float(S_OBJ))
        nc.vector.memset(nfull[:, 1:2], float(S_ROAD))
        wb = {}
        for seg, (wd, bd) in enumerate(((w_obj_d, b_obj_d), (w_road_d, b_road_d))):
            for h in range(2):
                wt = singles.tile([128, 1], F32, name=f"w{seg}{h}")
                nc.sync.dma_start(
                    out=wt, in_=bass.AP(tensor=wd, offset=128 * h, ap=[[1, 128], [1, 1]])
                )
                bt = singles.tile([128, 1], F32, name=f"b{seg}{h}")
                nc.sync.dma_start(
                    out=bt, in_=bass.AP(tensor=bd, offset=128 * h, ap=[[1, 128], [1, 1]])
                )
                wb[(seg, h)] = (wt, bt)

        xg_tiles = {}
        av_tiles = {}

        def emit_loads(c):
            av = chunkp.tile([CHUNK, S], BF16, name="av")
            nc.sync.dma_start(out=av, in_=alive_d[c * CHUNK : (c + 1) * CHUNK, :])
            av_tiles[c] = av
            for g in range(GPC):
                grp = c * GPC + g
                for h in range(2):
                    t = xp.tile([128, G, S], BF16, tag="xg", name="xg")
                    nc.sync.dma_start(out=t, in_=x_d[grp, 128 * h : 128 * (h + 1), :, :])
                    xg_tiles[(grp, h)] = t

        emit_loads(0)
        for c in range(NCHUNK):
            if c + 1 < NCHUNK:
                emit_loads(c + 1)

            # ---- count-derived rows per subchunk -> PE broadcast into Rps ----
            av = av_tiles.pop(c)
            Rps = psum.tile([128, 5, 2, 2, CHUNK], F32, tag="bank", name="Rps")
            for sub in range(NSUB):
                r0 = sub * SUB
                cnt = chunkp.tile([SUB, 2], F32, name=f"cnt{sub}")
                nc.vector.tensor_reduce(
                    cnt[:, 0:1], av[r0 : r0 + SUB, 0:S_OBJ], AX.X, OP.add
                )
                nc.vector.tensor_reduce(
                    cnt[:, 1:2], av[r0 : r0 + SUB, S_OBJ:S], AX.X, OP.add
                )
                cc = chunkp.tile([SUB, 2], F32, name=f"cc{sub}")
                rc = chunkp.tile([SUB, 2], F32, name=f"rc{sub}")
                pack = chunkp.tile([SUB, 32], F32, name=f"pack{sub}")
                nc.vector.tensor_scalar(cc, cnt, 1.0, None, OP.max)
                nc.vector.reciprocal(rc, cc)
                # rows: 0,1 = nr1 = nhalf*rcnt; 2,3 = g = nfull*rcnt - 2;
                #       4,5 = rcnt; 6,7 = okt; 8,9 = okm   (cols = obj, road)
                nc.vector.tensor_mul(pack[:, 0:2], rc, nhalf)
                nc.vector.tensor_mul(pack[:, 2:4], rc, nfull)
                nc.vector.tensor_scalar(pack[:, 2:4], pack[:, 2:4], -2.0, None, OP.add)
                nc.vector.tensor_scalar(pack[:, 4:6], rc, 1.0, None, OP.mult)
                nc.vector.tensor_scalar(pack[:, 6:8], cnt, -1.0, 1.0, OP.add, OP.min)
                nc.vector.tensor_scalar(pack[:, 6:8], pack[:, 6:8], 0.0, None, OP.max)
                nc.vector.tensor_scalar(
                    pack[:, 8:10], pack[:, 6:8], -1.0, 1.0, OP.mult, OP.add
                )
                packT = chunkp.tile([SUB, 32], F32, name=f"packT{sub}")
                nc.vector.transpose(packT, pack)
                packR = chunkp.tile([SUB, 32], F32R, name=f"packR{sub}")
                nc.vector.tensor_scalar(packR, packT, 1.0, None, OP.mult)
                for r in range(5):
                    for seg in range(2):
                        k = 2 * r + seg
                        for h in range(2):
                            nc.tensor.matmul(
                                Rps[:, r : r + 1, seg : seg + 1, h : h + 1,
                                    r0 : r0 + SUB],
                                SEL[:, 128 * k : 128 * (k + 1)],
                                packR[0:10, :],
                                start=True, stop=True,
                            )
            Rsb = chunkp.tile([128, 5, 2, 2, CHUNK], F32, name="Rsb")
            nc.vector.tensor_scalar(Rsb, Rps, 1.0, None, OP.mult)

            # ---- bn_stats over the set dim (d on partitions) ----
            BNo = bnp.tile([128, 2, CHUNK, 6], F32, tag="bno", name="BNo")
            BNr = bnp.tile([128, 2, CHUNK, 6], F32, tag="bnr", name="BNr")
            for g in range(GPC):
                grp = c * GPC + g
                for h in range(2):
                    xg = xg_tiles[(grp, h)]
                    for j in range(G):  # HW limit: one 6-tuple per op
                        bcol = G * g + j
                        nc.vector.bn_stats(
                            BNo[:, h : h + 1, bcol : bcol + 1, :],
                            xg[:, j : j + 1, 0:S_OBJ],
                        )
                        nc.vector.bn_stats(
                            BNr[:, h : h + 1, bcol : bcol + 1, :],
                            xg[:, j : j + 1, S_OBJ:S],
                        )

            # ---- stats chain per segment on [128, 2, 64] ----
            ACs = {}
            for seg, BN in ((0, BNo), (1, BNr)):
                me, Me = BN[:, :, :, 1:2], BN[:, :, :, 2:3]
                mo, Mo = BN[:, :, :, 4:5], BN[:, :, :, 5:6]

                def row(r, _seg=seg):
                    return Rsb[:, r : r + 1, _seg : _seg + 1, :, :]

                U = chunkp.tile([128, 2, CHUNK], F32, name=f"U{seg}")
                MEAN = chunkp.tile([128, 2, CHUNK], F32, name=f"MEAN{seg}")
                A1 = chunkp.tile([128, 2, CHUNK], F32, name=f"A1{seg}")
                B1 = chunkp.tile([128, 2, CHUNK], F32, name=f"B1{seg}")
                ISTD = chunkp.tile([128, 2, CHUNK], F32, name=f"ISTD{seg}")
                nc.vector.tensor_add(U, me, mo)            # me+mo
                nc.vector.tensor_mul(MEAN, U, row(0))      # mean = (me+mo)*nh*rcnt
                nc.vector.tensor_mul(A1, me, me)
                nc.vector.tensor_mul(B1, mo, mo)
                nc.vector.tensor_add(A1, A1, B1)           # me^2+mo^2
                nc.vector.tensor_add(U, Me, Mo)            # M2 sum
                nc.vector.tensor_mul(U, U, row(2))         # *rcnt
                nc.vector.tensor_mul(A1, A1, row(0))       # *nh*rcnt
                nc.vector.tensor_add(U, U, A1)             # s2/cnt
                nc.vector.tensor_mul(B1, MEAN, MEAN)       # mean^2
                nc.vector.tensor_mul(B1, B1, row(1))       # *(n*rcnt-2)
                nc.vector.tensor_add(U, U, B1)             # var
                nc.vector.tensor_scalar(U, U, EPS, None, OP.add)
                nc.vector.reciprocal(B1, U)                # 1/(var+eps)
                nc.scalar.activation(ISTD, B1, AF.Sqrt)    # istd
                nc.vector.tensor_mul(ISTD, ISTD, row(3))   # *okt
                nc.vector.tensor_add(ISTD, ISTD, row(4))   # +okm -> istd_eff
                for h in range(2):
                    wt, bt = wb[(seg, h)]
                    A_t = chunkp.tile([128, CHUNK], F32, name=f"A{seg}{h}")
                    C_t = chunkp.tile([128, CHUNK], F32, name=f"C{seg}{h}")
                    nc.vector.tensor_scalar(
                        A_t, ISTD[:, h : h + 1, :], wt[:, :], None, OP.mult
                    )
                    nc.vector.tensor_mul(C_t, MEAN[:, h : h + 1, :], A_t)
                    nc.vector.tensor_scalar(C_t, C_t, -1.0, bt[:, :], OP.mult, OP.add)
                    ACs[(seg, h)] = (A_t, C_t)

            # ---- apply in-place + out DMA ----
            for g in range(GPC):
                grp = c * GPC + g
                for h in range(2):
                    xg = xg_tiles.pop((grp, h))
                    Ao, Co = ACs[(0, h)]
                    Ar, Cr = ACs[(1, h)]
                    last = c == NCHUNK - 1
                    for j in range(G):
                        bcol = G * g + j
                        # obj: ScalarE (last chunk: half to DVE to kill tail)
                        if last and bcol % 2 == 1:
                            nc.vector.tensor_scalar(
                                xg[:, j : j + 1, 0:S_OBJ], xg[:, j : j + 1, 0:S_OBJ],
                                Ao[:, bcol : bcol + 1], Co[:, bcol : bcol + 1],
                                OP.mult, OP.add,
                            )
                        else:
                            nc.scalar.activation(
                                xg[:, j : j + 1, 0:S_OBJ], xg[:, j : j + 1, 0:S_OBJ],
                                AF.Identity,
                                bias=Co[:, bcol : bcol + 1],
                                scale=Ao[:, bcol : bcol + 1],
                            )
                        # road: split DVE / ScalarE
                        road_sc = 4 if last else ROAD_SC
                        if bcol % 16 < road_sc:
                            nc.scalar.activation(
                                xg[:, j : j + 1, S_OBJ:S], xg[:, j : j + 1, S_OBJ:S],
                                AF.Identity,
                                bias=Cr[:, bcol : bcol + 1],
                                scale=Ar[:, bcol : bcol + 1],
                            )
                        else:
                            nc.vector.tensor_scalar(
                                xg[:, j : j + 1, S_OBJ:S], xg[:, j : j + 1, S_OBJ:S],
                                Ar[:, bcol : bcol + 1], Cr[:, bcol : bcol + 1],
                                OP.mult, OP.add,
                            )
                    nc.gpsimd.dma_start(
                        out=out_d[grp, 128 * h : 128 * (h + 1), :, :], in_=xg[:, :, :]
                    )

    nc.compile()
    return nc


def _get_nc():
    if "nc" not in _NC_CACHE:
        _NC_CACHE["nc"] = build_nc()
    return _NC_CACHE["nc"]


def kernel(x, mask, weights_obj, biases_obj, weights_road, biases_road, _trace=False):
    x = np.asarray(x, dtype=np.float32)
    mask = np.asarray(mask).astype(bool)
    w_obj = np.ascontiguousarray(np.asarray(weights_obj, dtype=np.float32))
    b_obj = np.ascontiguousarray(np.asarray(biases_obj, dtype=np.float32))
    w_road = np.ascontiguousarray(np.asarray(weights_road, dtype=np.float32))
    b_road = np.ascontiguousarray(np.asarray(biases_road, dtype=np.float32))

    xm = np.where(mask[:, :, None], np.float32(0), x).astype(BF)
    # [8, NGRP, G, S, D] -> [8, NGRP, D, G, S]
    xt = np.ascontiguousarray(
        xm.reshape(NCORES, NGRP, G, S, D).transpose(0, 1, 4, 2, 3)
    )
    alive = (~mask).astype(BF).reshape(NCORES, B_LOC, S)
    sel = np.zeros((10, 10 * 128), dtype=np.float32)
    for r in range(10):
        sel[r, 128 * r : 128 * (r + 1)] = 1.0

    in_maps = [
        {
            "x": xt[i],
            "alive": alive[i],
            "sel": sel,
            "weights_obj": w_obj,
            "biases_obj": b_obj,
            "weights_road": w_road,
            "biases_road": b_road,
        }
        for i in range(NCORES)
    ]
    nc = _get_nc()
    res = run_bass_kernel_spmd(nc, in_maps, core_ids=list(range(NCORES)), trace=_trace)
    outs = []
    for i in range(NCORES):
        o = np.asarray(res.results[i]["out"])  # [NGRP, D, G, S] bf16
        outs.append(o.transpose(0, 2, 3, 1).reshape(B_LOC, S, D))
    out = np.concatenate(outs, axis=0).astype(np.float32)
    if _trace:
        kernel.last_exec_time_ns = res.exec_time_ns
        kernel.last_mean_exec_time_ns = res.mean_exec_time_ns
    return out.reshape(B, S, D)


# revision 17
# speedup vs baseline: 1.1879x; 1.0455x over previous
"""CrossSetNorm Trainium2 kernel v3 (8 NeuronCores, batch-parallel).

Problem: x [2048, 328, 256] f32, mask [2048, 328] bool (True = dead).
Two independent masked set-norms over the set dim per sample:
  obj = s in [0, 128), road = s in [128, 328)
out[s,d] = alive_s * x[s,d] * A[d] + C[d] with A = istd*w, C = b - mean*A.

Design (per core, 256 samples):
  - HOST: pre-mask xm = x*alive, cast bf16, transpose to [grp, d, samp, s]
    so the set dim s lands on the SBUF free axis and d on partitions.
    Output returns transposed bf16 and is un-transposed + cast on host.
  - stats: one DVE bn_stats per (sample, seg, d-half) gives (count, mean,
    M2) for even/odd elements -> s1/s2 recovered in a per-superchunk
    (64 samples) chain on [128, 2, 64] tiles (d on partitions,
    (half, sample) on free).
  - count-derived scalars (rcnt, okt, okm, ...) are computed per
    32-sample subchunk in [32, 2] sample-space, packed, stream-
    transposed, and broadcast to 128 partitions with one-hot-selector
    PE matmuls into PSUM.
  - istd = Exp(-0.5*Ln(var+eps)) on ScalarE; A/C tiles [128, 64] per
    (seg, half) hold per-sample columns.
  - apply: in-place per (sample, seg, half): DVE tensor_scalar
    (xm*A + C) or ScalarE activation Identity(scale=A, bias=C), split
    across both engines to balance load. Out-DMA per tile via gpsimd
    SWDGE; no gpsimd compute (the v1 bottleneck: gpsimd 89% busy).
"""
import sys

if "/opt/trn_rl_repo" not in sys.path:
    sys.path.insert(0, "/opt/trn_rl_repo")

from contextlib import ExitStack

import numpy as np
import ml_dtypes

import concourse.bacc as bacc
import concourse.bass as bass
import concourse.tile as tile
from concourse import mybir
from concourse.bass_utils import run_bass_kernel_spmd

F32 = mybir.dt.float32
F32R = mybir.dt.float32r
BF16 = mybir.dt.bfloat16
AF = mybir.ActivationFunctionType
OP = mybir.AluOpType
AX = mybir.AxisListType
BF = ml_dtypes.bfloat16

NCORES = 8
B, S, D = 2048, 328, 256
B_LOC = B // NCORES      # 256
S_OBJ = 128
S_ROAD = S - S_OBJ       # 200
G = 8                    # samples per xg tile
NGRP = B_LOC // G        # 32 groups per core
CHUNK = 32               # samples per stats chunk
SUB = 32                 # samples per count subchunk
NSUB = CHUNK // SUB      # 2
GPC = CHUNK // G         # 8 groups per superchunk
NCHUNK = B_LOC // CHUNK  # 4
EPS = 1e-6
# p3 road ops also routed to ScalarE when (bcol % 16) < ROAD_SC
ROAD_SC = 10

_NC_CACHE = {}


def build_nc():
    nc = bacc.Bacc("TRN2", target_bir_lowering=False, debug=False, num_devices=NCORES)
    x_d = nc.declare_dram_parameter("x", [NGRP, D, G, S], BF16, isOutput=False)
    alive_d = nc.declare_dram_parameter("alive", [B_LOC, S], BF16, isOutput=False)
    w_obj_d = nc.declare_dram_parameter("weights_obj", [D], F32, isOutput=False)
    b_obj_d = nc.declare_dram_parameter("biases_obj", [D], F32, isOutput=False)
    w_road_d = nc.declare_dram_parameter("weights_road", [D], F32, isOutput=False)
    b_road_d = nc.declare_dram_parameter("biases_road", [D], F32, isOutput=False)
    sel_d = nc.declare_dram_parameter("sel", [10, 10 * 128], F32R, isOutput=False)
    out_d = nc.declare_dram_parameter("out", [NGRP, D, G, S], BF16, isOutput=True)

    with tile.TileContext(nc) as tc, ExitStack() as ctx:
        singles = ctx.enter_context(tc.tile_pool(name="singles", bufs=1))
        chunkp = ctx.enter_context(tc.tile_pool(name="chunkp", bufs=2))
        xp = ctx.enter_context(tc.tile_pool(name="xp", bufs=24))
        bnp = ctx.enter_context(tc.tile_pool(name="bnp", bufs=2))
        psum = ctx.enter_context(tc.tile_pool(name="psum", bufs=2, space="PSUM"))

        # ---- constants ----
        eps_col = singles.tile([128, 1], F32)
        nc.vector.memset(eps_col, EPS)
        # one-hot selector: SEL[k, 128*r + m] = (k == r); lhsT slice r picks
        # packT row r and broadcasts it to 128 partitions (K=10 matmul)
        SEL = singles.tile([10, 10 * 128], F32R)
        nc.sync.dma_start(out=SEL, in_=sel_d[:, :])
        nhalf = singles.tile([SUB, 2], F32)  # even-element counts per seg
        nc.vector.memset(nhalf[:, 0:1], float(S_OBJ // 2))
        nc.vector.memset(nhalf[:, 1:2], float(S_ROAD // 2))
        nfull = singles.tile([SUB, 2], F32)  # segment sizes
        nc.vector.memset(nfull[:, 0:1], float(S_OBJ))
        nc.vector.memset(nfull[:, 1:2], float(S_ROAD))
        wb = {}
        for seg, (wd, bd) in enumerate(((w_obj_d, b_obj_d), (w_road_d, b_road_d))):
            for h in range(2):
                wt = singles.tile([128, 1], F32, name=f"w{seg}{h}")
                nc.sync.dma_start(
                    out=wt, in_=bass.AP(tensor=wd, offset=128 * h, ap=[[1, 128], [1, 1]])
                )
                bt = singles.tile([128, 1], F32, name=f"b{seg}{h}")
                nc.sync.dma_start(
                    out=bt, in_=bass.AP(tensor=bd, offset=128 * h, ap=[[1, 128], [1, 1]])
                )
                wb[(seg, h)] = (wt, bt)

        xg_tiles = {}
        av_tiles = {}

        def emit_loads(c):
            av = chunkp.tile([CHUNK, S], BF16, name="av")
            nc.sync.dma_start(out=av, in_=alive_d[c * CHUNK : (c + 1) * CHUNK, :])
            av_tiles[c] = av
            for g in range(GPC):
                grp = c * GPC + g
                for h in range(2):
                    t = xp.tile([128, G, S], BF16, tag="xg", name="xg")
                    nc.sync.dma_start(out=t, in_=x_d[grp, 128 * h : 128 * (h + 1), :, :])
                    xg_tiles[(grp, h)] = t

        emit_loads(0)
        for c in range(NCHUNK):
            if c + 1 < NCHUNK:
                emit_loads(c + 1)

            # ---- count-derived rows per subchunk -> PE broadcast into Rps ----
            av = av_tiles.pop(c)
            Rps = psum.tile([128, 5, 2, 2, CHUNK], F32, tag="bank", name="Rps")
            for sub in range(NSUB):
                r0 = sub * SUB
                cnt = chunkp.tile([SUB, 2], F32, name=f"cnt{sub}")
                nc.vector.tensor_reduce(
                    cnt[:, 0:1], av[r0 : r0 + SUB, 0:S_OBJ], AX.X, OP.add
                )
                nc.vector.tensor_reduce(
                    cnt[:, 1:2], av[r0 : r0 + SUB, S_OBJ:S], AX.X, OP.add
                )
                cc = chunkp.tile([SUB, 2], F32, name=f"cc{sub}")
                rc = chunkp.tile([SUB, 2], F32, name=f"rc{sub}")
                pack = chunkp.tile([SUB, 32], F32, name=f"pack{sub}")
                nc.vector.tensor_scalar(cc, cnt, 1.0, None, OP.max)
                nc.vector.reciprocal(rc, cc)
                # rows: 0,1 = nr1 = nhalf*rcnt; 2,3 = g = nfull*rcnt - 2;
                #       4,5 = rcnt; 6,7 = okt; 8,9 = okm   (cols = obj, road)
                nc.vector.tensor_mul(pack[:, 0:2], rc, nhalf)
                nc.vector.tensor_mul(pack[:, 2:4], rc, nfull)
                nc.vector.tensor_scalar(pack[:, 2:4], pack[:, 2:4], -2.0, None, OP.add)
                nc.vector.tensor_scalar(pack[:, 4:6], rc, 1.0, None, OP.mult)
                nc.vector.tensor_scalar(pack[:, 6:8], cnt, -1.0, 1.0, OP.add, OP.min)
                nc.vector.tensor_scalar(pack[:, 6:8], pack[:, 6:8], 0.0, None, OP.max)
                nc.vector.tensor_scalar(
                    pack[:, 8:10], pack[:, 6:8], -1.0, 1.0, OP.mult, OP.add
                )
                packT = chunkp.tile([SUB, 32], F32, name=f"packT{sub}")
                nc.vector.transpose(packT, pack)
                packR = chunkp.tile([SUB, 32], F32R, name=f"packR{sub}")
                nc.vector.tensor_scalar(packR, packT, 1.0, None, OP.mult)
                for r in range(5):
                    for seg in range(2):
                        k = 2 * r + seg
                        for h in range(2):
                            nc.tensor.matmul(
                                Rps[:, r : r + 1, seg : seg + 1, h : h + 1,
                                    r0 : r0 + SUB],
                                SEL[:, 128 * k : 128 * (k + 1)],
                                packR[0:10, :],
                                start=True, stop=True,
                            )
            Rsb = chunkp.tile([128, 5, 2, 2, CHUNK], F32, name="Rsb")
            nc.vector.tensor_scalar(Rsb, Rps, 1.0, None, OP.mult)

            # chunk 0 ramps up with two 16-sample blocks (2 groups each)
            blocks = [(0, 2), (2, 2)] if c == 0 else [(0, GPC)]
            for g0, ng in blocks:
                nb = ng * G  # samples in block
                b0 = g0 * G  # first bcol of block

                # ---- bn_stats over the set dim (d on partitions) ----
                BNo = bnp.tile([128, 2, nb, 6], F32, tag="bno", name=f"BNo{ng}")
                BNr = bnp.tile([128, 2, nb, 6], F32, tag="bnr", name=f"BNr{ng}")
                for g in range(g0, g0 + ng):
                    grp = c * GPC + g
                    for h in range(2):
                        xg = xg_tiles[(grp, h)]
                        for j in range(G):  # HW limit: one 6-tuple per op
                            lcol = G * (g - g0) + j
                            nc.vector.bn_stats(
                                BNo[:, h : h + 1, lcol : lcol + 1, :],
                                xg[:, j : j + 1, 0:S_OBJ],
                            )
                            nc.vector.bn_stats(
                                BNr[:, h : h + 1, lcol : lcol + 1, :],
                                xg[:, j : j + 1, S_OBJ:S],
                            )

                # ---- stats chain per segment on [128, 2, nb] ----
                ACs = {}
                for seg, BN in ((0, BNo), (1, BNr)):
                    me, Me = BN[:, :, :, 1:2], BN[:, :, :, 2:3]
                    mo, Mo = BN[:, :, :, 4:5], BN[:, :, :, 5:6]

                    def row(r, _seg=seg):
                        return Rsb[:, r : r + 1, _seg : _seg + 1, :, b0 : b0 + nb]

                    U = chunkp.tile([128, 2, nb], F32, name=f"U{seg}{ng}")
                    MEAN = chunkp.tile([128, 2, nb], F32, name=f"MEAN{seg}{ng}")
                    A1 = chunkp.tile([128, 2, nb], F32, name=f"A1{seg}{ng}")
                    B1 = chunkp.tile([128, 2, nb], F32, name=f"B1{seg}{ng}")
                    ISTD = chunkp.tile([128, 2, nb], F32, name=f"ISTD{seg}{ng}")
                    nc.vector.tensor_add(U, me, mo)            # me+mo
                    nc.vector.tensor_mul(MEAN, U, row(0))      # mean
                    nc.vector.tensor_mul(A1, me, me)
                    nc.vector.tensor_mul(B1, mo, mo)
                    nc.vector.tensor_add(A1, A1, B1)           # me^2+mo^2
                    nc.vector.tensor_add(U, Me, Mo)            # M2 sum
                    nc.vector.tensor_mul(U, U, row(2))         # *rcnt
                    nc.vector.tensor_mul(A1, A1, row(0))       # *nh*rcnt
                    nc.vector.tensor_add(U, U, A1)             # s2/cnt
                    nc.vector.tensor_mul(B1, MEAN, MEAN)       # mean^2
                    nc.vector.tensor_mul(B1, B1, row(1))       # *(n*rcnt-2)
                    nc.vector.tensor_add(U, U, B1)             # var
                    nc.vector.tensor_scalar(U, U, EPS, None, OP.add)
                    nc.vector.reciprocal(B1, U)                # 1/(var+eps)
                    nc.scalar.activation(ISTD, B1, AF.Sqrt)    # istd
                    nc.vector.tensor_mul(ISTD, ISTD, row(3))   # *okt
                    nc.vector.tensor_add(ISTD, ISTD, row(4))   # +okm -> istd_eff
                    for h in range(2):
                        wt, bt = wb[(seg, h)]
                        A_t = chunkp.tile([128, nb], F32, name=f"A{seg}{h}{ng}")
                        C_t = chunkp.tile([128, nb], F32, name=f"C{seg}{h}{ng}")
                        nc.vector.tensor_scalar(
                            A_t, ISTD[:, h : h + 1, :], wt[:, :], None, OP.mult
                        )
                        nc.vector.tensor_mul(C_t, MEAN[:, h : h + 1, :], A_t)
                        nc.vector.tensor_scalar(
                            C_t, C_t, -1.0, bt[:, :], OP.mult, OP.add
                        )
                        ACs[(seg, h)] = (A_t, C_t)

                # ---- apply in-place + out DMA ----
                for g in range(g0, g0 + ng):
                    grp = c * GPC + g
                    for h in range(2):
                        xg = xg_tiles.pop((grp, h))
                        Ao, Co = ACs[(0, h)]
                        Ar, Cr = ACs[(1, h)]
                        last = c == NCHUNK - 1
                        for j in range(G):
                            lcol = G * (g - g0) + j
                            bcol = G * g + j
                            o_sl = xg[:, j : j + 1, 0:S_OBJ]
                            # obj: ScalarE (last chunk: half to DVE)
                            if last and bcol % 2 == 1:
                                nc.vector.tensor_scalar(
                                    o_sl, o_sl,
                                    Ao[:, lcol : lcol + 1], Co[:, lcol : lcol + 1],
                                    OP.mult, OP.add,
                                )
                            else:
                                nc.scalar.activation(
                                    o_sl, o_sl,
                                    AF.Identity,
                                    bias=Co[:, lcol : lcol + 1],
                                    scale=Ao[:, lcol : lcol + 1],
                                )
                            # road: split DVE / ScalarE
                            r_sl = xg[:, j : j + 1, S_OBJ:S]
                            road_sc = 4 if last else ROAD_SC
                            if bcol % 16 < road_sc:
                                nc.scalar.activation(
                                    r_sl, r_sl,
                                    AF.Identity,
                                    bias=Cr[:, lcol : lcol + 1],
                                    scale=Ar[:, lcol : lcol + 1],
                                )
                            else:
                                nc.vector.tensor_scalar(
                                    r_sl, r_sl,
                                    Ar[:, lcol : lcol + 1], Cr[:, lcol : lcol + 1],
                                    OP.mult, OP.add,
                                )
                        nc.gpsimd.dma_start(
                            out=out_d[grp, 128 * h : 128 * (h + 1), :, :],
                            in_=xg[:, :, :],
                        )

    nc.compile()
    return nc


def _get_nc():
    if "nc" not in _NC_CACHE:
        _NC_CACHE["nc"] = build_nc()
    return _NC_CACHE["nc"]


def kernel(x, mask, weights_obj, biases_obj, weights_road, biases_road, _trace=False):
    x = np.asarray(x, dtype=np.float32)
    mask = np.asarray(mask).astype(bool)
    w_obj = np.ascontiguousarray(np.asarray(weights_obj, dtype=np.float32))
    b_obj = np.ascontiguousarray(np.asarray(biases_obj, dtype=np.float32))
    w_road = np.ascontiguousarray(np.asarray(weights_road, dtype=np.float32))
    b_road = np.ascontiguousarray(np.asarray(biases_road, dtype=np.float32))

    xm = np.where(mask[:, :, None], np.float32(0), x).astype(BF)
    # [8, NGRP, G, S, D] -> [8, NGRP, D, G, S]
    xt = np.ascontiguousarray(
        xm.reshape(NCORES, NGRP, G, S, D).transpose(0, 1, 4, 2, 3)
    )
    alive = (~mask).astype(BF).reshape(NCORES, B_LOC, S)
    sel = np.zeros((10, 10 * 128), dtype=np.float32)
    for r in range(10):
        sel[r, 128 * r : 128 * (r + 1)] = 1.0

    in_maps = [
        {
            "x": xt[i],
            "alive": alive[i],
            "sel": sel,
            "weights_obj": w_obj,
            "biases_obj": b_obj,
            "weights_road": w_road,
            "biases_road": b_road,
        }
        for i in range(NCORES)
    ]
    nc = _get_nc()
    res = run_bass_kernel_spmd(nc, in_maps, core_ids=list(range(NCORES)), trace=_trace)
    outs = []
    for i in range(NCORES):
        o = np.asarray(res.results[i]["out"])  # [NGRP, D, G, S] bf16
        outs.append(o.transpose(0, 2, 3, 1).reshape(B_LOC, S, D))
    out = np.concatenate(outs, axis=0).astype(np.float32)
    if _trace:
        kernel.last_exec_time_ns = res.exec_time_ns
        kernel.last_mean_exec_time_ns = res.mean_exec_time_ns
    return out.reshape(B, S, D)
